# revision 3
# baseline (speedup 1.0000x reference)
"""Trainium2 Bass kernel for nn_ChordalPCWeightTransform.

Math: the reference does
    out = softmax( P_orig( P_rootfirst(x) * w ), axis=-1 )
where P_rootfirst / P_orig are per-label rolls of the first 12 pitch
classes (last slot fixed).  The two permutations are exact inverses, so
the whole transform collapses to
    out[b, l, :] = softmax( x[b, l, :] * W[l, :] )
with W[l, j] = w[(j - root_pc(l)) % 12] for j < 12 and W[l, 12] = w[12].
W ([144, 13]) is a cheap host-side gather of the 13 learned weights.

v2: bf16 I/O (tolerance is 2e-2; bf16 keeps us ~1e-2) halves HBM traffic
vs the f32 baseline (123 MB -> 61 MB per core).  That moves the memory
roofline from ~343us to ~171us, so compute must be spread across engines:
  DMA in (HWDGE/sync, bf16) ->
  DVE: t = x * W           (bf16 dense tensor_tensor, 2x_1P mode)
  ACT: ep[.., 0:13] = exp(t)  strided into a 16-wide padded tile
       (pad lanes memset to 0 once; exp never writes them)
  DVE: s = pairwise-add tree over the 16 lanes (L1/L2 bf16 at 2x,
       L3/L4 f32 at 1x) -- cheaper than 1x tensor_reduce over 13
  ACT: ls = ln(s); r = exp(-ls) = 1/s   (same ACT table set as exp)
  DVE/GPSIMD (split by tile): out = e * r (broadcast over 13, 1x)
  DMA out (HWDGE/scalar, bf16)
Host upcasts the bf16 result to f32.
"""

import numpy as np
import ml_dtypes

import concourse.bass as bass
import concourse.bacc as bacc
import concourse.tile as tile
from concourse import mybir
from concourse.bass_utils import run_bass_kernel_spmd

B, L, P = 65536, 144, 13
NCORES = 8
BS = B // NCORES   # 8192 frames per core
ROW = L * P        # 1872 floats per frame
TP = 128           # SBUF partitions
FPB = 2            # frames per partition per tile
TFREE = FPB * ROW  # free-dim elems per tile (x / out tiles)
K = FPB * L        # softmax groups per partition per tile (288)
PAD = 16           # padded group width for the reduction tree
NEP = 4            # ep (padded exp) tiles, rotated manually

# Of every DVE_FINAL_EVERY tiles, DVE does 1 final broadcast-mult and
# GPSIMD does the rest.  0 means GPSIMD does all of them.
DVE_FINAL_EVERY = 4

F32 = mybir.dt.float32
BF16 = mybir.dt.bfloat16


def _build_weight_table(w: np.ndarray) -> np.ndarray:
    """Effective per-label weight table W[l, j] = w[idx_original[l, j]]."""
    num_quality = L // 12
    root_pc = np.arange(L) // num_quality
    n = P - 1
    j = np.arange(n)
    idx12 = (j[None, :] - root_pc[:, None]) % n
    idx = np.concatenate([idx12, np.full((L, 1), n, dtype=idx12.dtype)], axis=1)
    return np.ascontiguousarray(w.astype(np.float32)[idx])  # [144, 13]


def _pin_act_table(nc) -> None:
    """Make Exp and Ln resolvable only from the combined set so Bacc emits a
    single ACT_TABLE_LOAD instead of thrashing exp<->ln sets every tile."""
    from concourse.hw_specs import get_activation_tables

    tabs = get_activation_tables(nc.m.arch)
    keep = "natural_log_exp_and_others"
    if keep not in tabs:
        return
    exp = mybir.ActivationFunctionType.Exp
    ln = mybir.ActivationFunctionType.Ln
    for name, fns in tabs.items():
        if name != keep:
            fns.discard(exp)
            fns.discard(ln)


def build_module(n_frames: int = BS) -> bass.Bass:
    tile_frames = TP * FPB
    assert n_frames % tile_frames == 0
    nt = n_frames // tile_frames
    nc = bacc.Bacc()
    _pin_act_table(nc)
    x_in = nc.declare_dram_parameter("x", [n_frames, ROW], BF16, isOutput=False)
    w_in = nc.declare_dram_parameter("w", [ROW], BF16, isOutput=False)
    y_out = nc.declare_dram_parameter("y", [n_frames, ROW], BF16, isOutput=True)
    # Per-tile view: partition p holds FPB consecutive frames, contiguous.
    x_v = x_in.rearrange("(n p f) r -> n p (f r)", p=TP, f=FPB)
    y_v = y_out.rearrange("(n p f) r -> n p (f r)", p=TP, f=FPB)

    with tile.TileContext(nc) as tc:
        with (
            tc.tile_pool(name="singles", bufs=1) as singles,
            tc.tile_pool(name="xin", bufs=4) as xpool,
            tc.tile_pool(name="yout", bufs=4) as ypool,
            tc.tile_pool(name="tree", bufs=2) as tpool,
            tc.tile_pool(name="stats", bufs=2) as spool,
        ):
            # W row replicated across partitions and FPB frame slots.
            wb = singles.tile([TP, TFREE], BF16)
            nc.gpsimd.dma_start(
                out=wb[:],
                in_=w_in[None, None, :].to_broadcast([TP, FPB, ROW]),
            )

            # Padded exp tiles [TP, K, 16].  Pad lanes (13:16) must be 0 so
            # the 16-wide add tree computes the 13-wide group sum.  exp only
            # ever writes lanes 0:13, so one memset up front is enough.
            eps = []
            for j in range(NEP):
                ep = singles.tile(
                    [TP, K * PAD], BF16, name=f"ep{j}", tag=f"ep{j}"
                )
                nc.vector.memset(ep[:], 0.0)
                eps.append(ep)

            for i in range(nt):
                x_t = xpool.tile([TP, TFREE], BF16)
                nc.sync.dma_start(out=x_t[:], in_=x_v[i])

                # t = x * W  (bf16 dense, DVE 2x_1P)
                nc.vector.tensor_tensor(
                    out=x_t[:], in0=x_t[:], in1=wb[:], op=mybir.AluOpType.mult
                )

                # e = exp(t), written strided into lanes 0:13 of the padded
                # tile (ACT is 1x regardless of strides).
                ep = eps[i % NEP]
                ep3 = ep.rearrange("p (g d) -> p g d", d=PAD)
                x3 = x_t.rearrange("p (g d) -> p g d", d=P)
                nc.scalar.activation(
                    out=ep3[:, :, 0:P], in_=x3,
                    func=mybir.ActivationFunctionType.Exp,
                )

                # Pairwise-add tree over the 16 lanes:
                # L1/L2 in bf16 (2x mode), L3/L4 in f32 (1x) for accuracy.
                a_t = tpool.tile([TP, K * 8], BF16)
                a3 = a_t.rearrange("p (g d) -> p g d", d=8)
                nc.vector.tensor_tensor(
                    out=a3, in0=ep3[:, :, 0:8], in1=ep3[:, :, 8:16],
                    op=mybir.AluOpType.add,
                )
                b_t = tpool.tile([TP, K * 4], BF16)
                b3 = b_t.rearrange("p (g d) -> p g d", d=4)
                nc.vector.tensor_tensor(
                    out=b3, in0=a3[:, :, 0:4], in1=a3[:, :, 4:8],
                    op=mybir.AluOpType.add,
                )
                c_t = tpool.tile([TP, K * 2], F32)
                c3 = c_t.rearrange("p (g d) -> p g d", d=2)
                nc.vector.tensor_tensor(
                    out=c3, in0=b3[:, :, 0:2], in1=b3[:, :, 2:4],
                    op=mybir.AluOpType.add,
                )
                s_t = spool.tile([TP, K], F32)
                nc.vector.tensor_tensor(
                    out=s_t[:], in0=c3[:, :, 0], in1=c3[:, :, 1],
                    op=mybir.AluOpType.add,
                )

                # ls = ln(s); r = exp(-ls) = 1/s  (same ACT table set)
                nc.scalar.activation(
                    out=s_t[:], in_=s_t[:],
                    func=mybir.ActivationFunctionType.Ln,
                )
                r_t = spool.tile([TP, K], F32)
                nc.scalar.activation(
                    out=r_t[:], in_=s_t[:],
                    func=mybir.ActivationFunctionType.Exp, scale=-1.0,
                )

                # out = e * r (broadcast over the 13 lanes -> 1x on DVE).
                # Split between DVE and GPSIMD to balance engine load.
                y_t = ypool.tile([TP, TFREE], BF16)
                y3 = y_t.rearrange("p (g d) -> p g d", d=P)
                r_b = r_t[:, :, None].to_broadcast([TP, K, P])
                if DVE_FINAL_EVERY and i % DVE_FINAL_EVERY == 0:
                    eng = nc.vector
                else:
                    eng = nc.gpsimd
                eng.tensor_tensor(
                    out=y3, in0=ep3[:, :, 0:P], in1=r_b,
                    op=mybir.AluOpType.mult,
                )

                nc.scalar.dma_start(out=y_v[i], in_=y_t[:])

    nc.finalize()
    return nc


_MODULE_CACHE: dict[int, bass.Bass] = {}


def _get_module(n_frames: int = BS) -> bass.Bass:
    if n_frames not in _MODULE_CACHE:
        _MODULE_CACHE[n_frames] = build_module(n_frames)
    return _MODULE_CACHE[n_frames]


def make_in_maps(x: np.ndarray, w: np.ndarray) -> list[dict[str, np.ndarray]]:
    weff = _build_weight_table(w).reshape(ROW).astype(ml_dtypes.bfloat16)
    xb = np.ascontiguousarray(x.reshape(B, ROW)).astype(ml_dtypes.bfloat16)
    return [
        {"x": xb[i * BS : (i + 1) * BS], "w": weff}
        for i in range(NCORES)
    ]


def kernel(**inputs: np.ndarray) -> np.ndarray:
    x = np.asarray(inputs["chordal_pc_vector"], dtype=np.float32)
    w = np.asarray(inputs["scale_degree_weight"], dtype=np.float32)
    assert x.shape == (B, L, P), x.shape

    nc = _get_module()
    in_maps = make_in_maps(x, w)
    res = run_bass_kernel_spmd(nc, in_maps, core_ids=list(range(NCORES)))
    out = np.concatenate(
        [
            np.asarray(res.results[i]["y"]).astype(np.float32).reshape(BS, L, P)
            for i in range(NCORES)
        ],
        axis=0,
    )
    return out


# revision 4
# speedup vs baseline: 1.1866x; 1.1866x over previous
"""Trainium2 Bass kernel for nn_ChordalPCWeightTransform.

Math: the reference does
    out = softmax( P_orig( P_rootfirst(x) * w ), axis=-1 )
where P_rootfirst / P_orig are per-label rolls of the first 12 pitch
classes (last slot fixed).  The two permutations are exact inverses, so
the whole transform collapses to
    out[b, l, :] = softmax( x[b, l, :] * W[l, :] )
with W[l, j] = w[(j - root_pc(l)) % 12] for j < 12 and W[l, 12] = w[12].
W ([144, 13]) is a cheap host-side gather of the 13 learned weights.

v3: bf16 I/O (tolerance is 2e-2; bf16 keeps us ~1e-2) halves HBM traffic
vs the f32 baseline (123 MB -> 61 MB per core), moving the memory
roofline from ~343us to ~171us.  Compute is spread across engines with
measured rates (DVE 2x for dense bf16 tensor_tensor, 1x for reduce and
broadcast ops; GPSIMD ~2.5 cyc/elem):
  DMA in (HWDGE/sync, bf16)
  DVE:  t = x * W          (bf16 dense, 2x_1P)
  ACT:  e = exp(t)         (bf16 out)
  DVE:  s = reduce_sum over 13  (f32 out, 1x -- single op beats an add
        tree because every extra DVE op pays a ~0.4us DRAIN)
  ACT:  ls = ln(s); r = exp(-ls) = 1/s   (same ACT table set as exp)
  GPSIMD (6 of 7 tiles) / DVE (1 of 7): out = e * r (broadcast over 13)
  DMA out (HWDGE/scalar, bf16)
Host upcasts the bf16 result to f32.
"""

import numpy as np
import ml_dtypes

import concourse.bass as bass
import concourse.bacc as bacc
import concourse.tile as tile
from concourse import mybir
from concourse.bass_utils import run_bass_kernel_spmd

B, L, P = 65536, 144, 13
NCORES = 8
BS = B // NCORES   # 8192 frames per core
ROW = L * P        # 1872 floats per frame
TP = 128           # SBUF partitions
FPB = 4            # frames per partition per tile
TFREE = FPB * ROW  # free-dim elems per tile (x / e / out tiles)
K = FPB * L        # softmax groups per partition per tile (576)

# Of every DVE_FINAL_EVERY tiles, DVE does 1 final broadcast-mult and
# GPSIMD does the rest.  0 means GPSIMD does all of them.
DVE_FINAL_EVERY = 7

F32 = mybir.dt.float32
BF16 = mybir.dt.bfloat16


def _build_weight_table(w: np.ndarray) -> np.ndarray:
    """Effective per-label weight table W[l, j] = w[idx_original[l, j]]."""
    num_quality = L // 12
    root_pc = np.arange(L) // num_quality
    n = P - 1
    j = np.arange(n)
    idx12 = (j[None, :] - root_pc[:, None]) % n
    idx = np.concatenate([idx12, np.full((L, 1), n, dtype=idx12.dtype)], axis=1)
    return np.ascontiguousarray(w.astype(np.float32)[idx])  # [144, 13]


def _pin_act_table(nc) -> None:
    """Make Exp and Ln resolvable only from the combined set so Bacc emits a
    single ACT_TABLE_LOAD instead of thrashing exp<->ln sets every tile."""
    from concourse.hw_specs import get_activation_tables

    tabs = get_activation_tables(nc.m.arch)
    keep = "natural_log_exp_and_others"
    if keep not in tabs:
        return
    exp = mybir.ActivationFunctionType.Exp
    ln = mybir.ActivationFunctionType.Ln
    for name, fns in tabs.items():
        if name != keep:
            fns.discard(exp)
            fns.discard(ln)


def build_module(n_frames: int = BS) -> bass.Bass:
    tile_frames = TP * FPB
    assert n_frames % tile_frames == 0
    nt = n_frames // tile_frames
    nc = bacc.Bacc()
    _pin_act_table(nc)
    x_in = nc.declare_dram_parameter("x", [n_frames, ROW], BF16, isOutput=False)
    w_in = nc.declare_dram_parameter("w", [ROW], BF16, isOutput=False)
    y_out = nc.declare_dram_parameter("y", [n_frames, ROW], BF16, isOutput=True)
    # Per-tile view: partition p holds FPB consecutive frames, contiguous.
    x_v = x_in.rearrange("(n p f) r -> n p (f r)", p=TP, f=FPB)
    y_v = y_out.rearrange("(n p f) r -> n p (f r)", p=TP, f=FPB)

    with tile.TileContext(nc) as tc:
        with (
            tc.tile_pool(name="singles", bufs=1) as singles,
            tc.tile_pool(name="xin", bufs=3) as xpool,
            tc.tile_pool(name="etile", bufs=3) as epool,
            tc.tile_pool(name="yout", bufs=3) as ypool,
            tc.tile_pool(name="stats", bufs=3) as spool,
        ):
            # W row replicated across partitions and FPB frame slots.
            wb = singles.tile([TP, TFREE], BF16)
            nc.gpsimd.dma_start(
                out=wb[:],
                in_=w_in[None, None, :].to_broadcast([TP, FPB, ROW]),
            )

            for i in range(nt):
                x_t = xpool.tile([TP, TFREE], BF16)
                nc.sync.dma_start(out=x_t[:], in_=x_v[i])

                # t = x * W  (bf16 dense, DVE 2x_1P)
                nc.vector.tensor_tensor(
                    out=x_t[:], in0=x_t[:], in1=wb[:], op=mybir.AluOpType.mult
                )

                # e = exp(t)
                e_t = epool.tile([TP, TFREE], BF16)
                nc.scalar.activation(
                    out=e_t[:], in_=x_t[:],
                    func=mybir.ActivationFunctionType.Exp,
                )

                # s[p, g] = sum_j e[p, g, j]  (DVE 1x, f32 accum/out)
                e3 = e_t.rearrange("p (g d) -> p g d", d=P)
                s_t = spool.tile([TP, K], F32)
                nc.vector.reduce_sum(
                    out=s_t[:], in_=e3, axis=mybir.AxisListType.X
                )

                # ls = ln(s); r = exp(-ls) = 1/s  (same ACT table set)
                nc.scalar.activation(
                    out=s_t[:], in_=s_t[:],
                    func=mybir.ActivationFunctionType.Ln,
                )
                r_t = spool.tile([TP, K], F32)
                nc.scalar.activation(
                    out=r_t[:], in_=s_t[:],
                    func=mybir.ActivationFunctionType.Exp, scale=-1.0,
                )

                # out = e * r (broadcast over the 13 lanes -> 1x on DVE).
                # Split between GPSIMD (most tiles) and DVE to balance load.
                y_t = ypool.tile([TP, TFREE], BF16)
                y3 = y_t.rearrange("p (g d) -> p g d", d=P)
                r_b = r_t[:, :, None].to_broadcast([TP, K, P])
                if DVE_FINAL_EVERY and i % DVE_FINAL_EVERY == 0:
                    eng = nc.vector
                else:
                    eng = nc.gpsimd
                eng.tensor_tensor(
                    out=y3, in0=e3, in1=r_b, op=mybir.AluOpType.mult
                )

                nc.scalar.dma_start(out=y_v[i], in_=y_t[:])

    nc.finalize()
    return nc


_MODULE_CACHE: dict[int, bass.Bass] = {}


def _get_module(n_frames: int = BS) -> bass.Bass:
    if n_frames not in _MODULE_CACHE:
        _MODULE_CACHE[n_frames] = build_module(n_frames)
    return _MODULE_CACHE[n_frames]


def make_in_maps(x: np.ndarray, w: np.ndarray) -> list[dict[str, np.ndarray]]:
    weff = _build_weight_table(w).reshape(ROW).astype(ml_dtypes.bfloat16)
    xb = np.ascontiguousarray(x.reshape(B, ROW)).astype(ml_dtypes.bfloat16)
    return [
        {"x": xb[i * BS : (i + 1) * BS], "w": weff}
        for i in range(NCORES)
    ]


def kernel(**inputs: np.ndarray) -> np.ndarray:
    x = np.asarray(inputs["chordal_pc_vector"], dtype=np.float32)
    w = np.asarray(inputs["scale_degree_weight"], dtype=np.float32)
    assert x.shape == (B, L, P), x.shape

    nc = _get_module()
    in_maps = make_in_maps(x, w)
    res = run_bass_kernel_spmd(nc, in_maps, core_ids=list(range(NCORES)))
    out = np.concatenate(
        [
            np.asarray(res.results[i]["y"]).astype(np.float32).reshape(BS, L, P)
            for i in range(NCORES)
        ],
        axis=0,
    )
    return out
